# revision 41
# baseline (speedup 1.0000x reference)
"""Trainium2 Bass kernel for a DGCNN-style point-cloud encoder.

Per batch element (one per NeuronCore, B=8): kNN graph (k=20) over N=4096
points via a distance matmul + iterative top-8 extraction (max/max_index/
match_replace), edge-feature MLP with two training-mode batchnorms (global
stats via cross-core AllReduce) and leaky-relu, then max-pool over
neighbors.  Layout strategy: the first MLP layer is decomposed into
per-point projections A = W1a@p and C = (W1b-W1a)@p; the gather of A rows
by neighbor index runs as an indirect DMA with compute_op=add onto a
C-prefilled tile, so h1_pre arrives in one pass; PE transposes pairs of
neighbors into channelx2 PSUM tiles for the W2 stage.
"""
import sys
sys.path.insert(0, '/opt/trn_rl_repo')

import numpy as np
import orjson

import concourse.bass as bass
import concourse.mybir as mybir
import concourse.tile as tile
from concourse import library_config
from concourse.bass_utils import run_bass_kernel_spmd

# ---------------------------------------------------------------------------
# Workaround for walrus 'Too many sync wait commands': this toolchain accepts
# at most one sem-wait per lowered instruction. Split any instruction carrying
# more waits into EventSemaphore wait-carriers placed immediately before it.
# ---------------------------------------------------------------------------
_MAXW = 1


def _split_excess_waits(j) -> bool:
    changed = False
    for fn in j.get("functions", []):
        for blk in fn.get("blocks", []):
            out = []
            for inst in blk.get("instructions", []):
                si = inst.get("sync_info") or {}
                ow = si.get("on_wait") or []
                if len(ow) > _MAXW:
                    changed = True
                    chunks = [ow[i:i + _MAXW] for i in range(0, len(ow), _MAXW)]
                    for ci, chunk in enumerate(chunks[:-1]):
                        out.append({
                            "debug": inst.get("debug", 0),
                            "engine": inst["engine"],
                            "ins": [], "outs": [],
                            "name": f"{inst['name']}-w{ci}",
                            "opcode": "EventSemaphore",
                            "sync_info": {"on_update": [], "on_wait": chunk},
                        })
                    si = dict(si)
                    si["on_wait"] = chunks[-1]
                    inst = dict(inst)
                    inst["sync_info"] = si
                out.append(inst)
            blk["instructions"] = out
    return changed


_orig_to_json_bytes = bass.Bass.to_json_bytes


def _patched_to_json_bytes(self) -> bytes:
    raw = _orig_to_json_bytes(self)
    j = orjson.loads(raw)
    if _split_excess_waits(j):
        return orjson.dumps(j)
    return raw


bass.Bass.to_json_bytes = _patched_to_json_bytes

# ---------------------------------------------------------------------------
# Problem constants (hardcoded; kernel.py must be self-contained)
# ---------------------------------------------------------------------------
B = 8            # batch = number of cores
N = 4096         # points per cloud
KNN = 20         # neighbors
CH = 64          # hidden channels
EPS = 1e-5
ALPHA = 0.2      # leaky-relu slope
NM = N // 128    # 32 row-tiles
NJ2 = KNN // 2   # 10 neighbor pairs
CNT = B * N * KNN  # batchnorm population size (global over all cores)
NEG = -1.0e30
PGROUPS = [(0, 4), (4, 8), (8, 10)]     # j2 pair-groups per psum tile
WCHUNKS = [(0, 512), (512, 1024), (1024, 1280)]  # W2 rhs chunks per parity

# --- packed top-k constants -------------------------------------------------
# Scores are quantized per-row to an 11-bit field with the 12-bit global
# column index packed into the low mantissa bits: after the scalar-engine
# affine (score*beta_row + bias_row + MAGIC), fp32 rounding at exponent 23
# quantizes to integers; subtracting MAGIC and adding j/4096 yields
# packed = q + j/4096 in [2048, 4096) whose low 12 mantissa bits are j.
# beta_row = (S0/CCLAMP)*exp(-|p_i|^2/3) adapts the clamp window to the
# local point density (validated: max_row d24^2/exp(sq/3) = 0.157 < 0.22).
MAGIC = 12582912.0          # 1.5 * 2^23
BIAS0 = 4050.0
S0 = 2002.0
CCLAMP = 0.22
LNB0 = 9.116029692504942    # ln(S0 / CCLAMP)

f32 = mybir.dt.float32
f16 = mybir.dt.float16
u32 = mybir.dt.uint32
u16 = mybir.dt.uint16
i16 = mybir.dt.int16
i32 = mybir.dt.int32
ACTF = mybir.ActivationFunctionType


def _bcast_mid(ap, reps):
    """Insert a step-0 dim after the partition dim: [P, F] -> [P, reps, F]."""
    return bass.AP(ap.tensor, ap.offset,
                   [list(ap.ap[0]), [0, reps], list(ap.ap[1])])


def _build_program():
    nc = bass.Bass("TRN2", target_bir_lowering=False, debug=False,
                   num_devices=B)

    xb = nc.dram_tensor("xb", [4, N], f32, kind="ExternalInput")
    jvt = nc.dram_tensor("jv", [128, N], f32, kind="ExternalInput")
    sqd = nc.dram_tensor("sqd", [N, 1], f32)
    w1g = nc.dram_tensor("w1g", [80, NJ2 * 128], f16, kind="ExternalInput")
    pts16 = nc.dram_tensor("pts16", [N, 4], f16, kind="ExternalInput")
    w2t = nc.dram_tensor("w2t", [128, CH], f16, kind="ExternalInput")
    bn1g = nc.dram_tensor("bn1g", [CH, 1], f32, kind="ExternalInput")
    bn1b = nc.dram_tensor("bn1b", [CH, 1], f32, kind="ExternalInput")
    bn2g = nc.dram_tensor("bn2g", [CH, 1], f32, kind="ExternalInput")
    bn2b = nc.dram_tensor("bn2b", [CH, 1], f32, kind="ExternalInput")
    out_t = nc.dram_tensor("out", [CH, N], f32, kind="ExternalOutput")

    cc1_in = nc.dram_tensor("cc1_in", [128, 2], f32)
    cc1_out = nc.dram_tensor("cc1_out", [128, 2], f32, addr_space="Shared")
    cc2_in = nc.dram_tensor("cc2_in", [CH, 2], f32)
    cc2_out = nc.dram_tensor("cc2_out", [CH, 2], f32, addr_space="Shared")
    groups = [list(range(B))]

    with tile.TileContext(nc) as tc:
        const = tc.alloc_tile_pool(name="const", bufs=1)
        dramp = tc.alloc_tile_pool(name="dram", bufs=1, space="DRAM")
        abpool = tc.alloc_tile_pool(name="ab", bufs=1)

        # whole-kernel tensors
        w2t_sb = const.tile([128, CH], f16)
        g1_sb = const.tile([CH, 1], f32)
        b1in_sb = const.tile([CH, 1], f32)
        g2_sb = const.tile([CH, 1], f32)
        b2in_sb = const.tile([CH, 1], f32)
        h1p = const.tile([128, NM * NJ2 * 128], f16)   # stored h1_pre
        a1_sb = const.tile([128, 1], f32)
        b1_sb = const.tile([128, 1], f32)
        a2_sb = const.tile([CH, 1], f32)
        b2_sb = const.tile([CH, 1], f32)
        s2c = const.tile([CH, NM * 2], f32)            # phaseC sum slots
        s2q = const.tile([CH, NM * 2], f32)            # phaseC sum-sq slots
        jv_sb = const.tile([128, N], f32)              # j/4096 index column vec

        # phase A/B tensors (released after phase AB)
        # K=12 concatenated split operands: [hi;hi;lo] x [hi;lo;hi] computes
        # hi*hi + hi*lo + lo*hi in ONE matmul (fp32 to ~2^-24 rel)
        lhs_cat = abpool.tile([12, N], f16)
        rhs_cat = abpool.tile([12, N], f16)
        w1g_sb = abpool.tile([80, NJ2 * 128], f16)
        s1sum = abpool.tile([128, 3 * NM], f32)
        s1sq = abpool.tile([128, 3 * NM], f32)

        nc.sync.dma_start(w1g_sb[:], w1g.ap())
        nc.sync.dma_start(w2t_sb[:], w2t.ap())
        nc.sync.dma_start(g1_sb[:], bn1g.ap())
        nc.sync.dma_start(b1in_sb[:], bn1b.ap())
        nc.sync.dma_start(g2_sb[:], bn2g.ap())
        nc.sync.dma_start(b2in_sb[:], bn2b.ap())
        nc.sync.dma_start(jv_sb[:], jvt.ap())

        with nc.named_scope("stage0"):
            with tc.tile_pool(name="s0", bufs=1) as s0big, \
                 tc.tile_pool(name="s0s", bufs=1) as s0pool, \
                 tc.tile_pool(name="s0ps", bufs=2, space="PSUM") as s0psum:
                lhs_all = s0big.tile([4, N], f32, tag="lhs")
                rhs_all = s0big.tile([4, N], f32, tag="rhs")
                nc.sync.dma_start(lhs_all[:], xb.ap())
                nc.scalar.mul(rhs_all[0:3, :], lhs_all[0:3, :], 2.0)
                xsq = s0big.tile([4, N], f32, tag="scratch")
                nc.scalar.square(xsq[0:3, :], lhs_all[0:3, :])
                ones3 = nc.const_aps.tensor(1.0, (3, 1), f32)
                for j in range(N // 512):
                    ps = s0psum.tile([1, 512], f32, space="PSUM", tag="sq")
                    nc.tensor.matmul(ps[:], lhsT=ones3,
                                     rhs=xsq[0:3, bass.ts(j, 512)],
                                     start=True, stop=True)
                    sqneg = s0pool.tile([1, 512], f32, tag="sqneg")
                    nc.scalar.activation(sqneg[:], ps[:], ACTF.Copy, scale=-1.0)
                    nc.sync.dma_start(rhs_all[3:4, bass.ts(j, 512)], sqneg[:])
                nc.sync.dma_start(sqd.ap(), rhs_all[3:4, :])
                # fp16 hi/lo split of lhs/rhs for single-pass PE matmuls
                for full, cat, pattern in ((lhs_all, lhs_cat, (0, 0, 1)),
                                           (rhs_all, rhs_cat, (0, 1, 0))):
                    hi = s0pool.tile([4, N], f16, tag="hi")
                    nc.scalar.copy(hi[:], full[:])
                    rb = s0big.tile([4, N], f32, tag="scratch")
                    nc.vector.tensor_sub(rb[:], full[:], hi[:])
                    lo = s0pool.tile([4, N], f16, tag="lo")
                    nc.scalar.copy(lo[:], rb[:])
                    for slot, which in enumerate(pattern):
                        nc.sync.dma_start(cat[slot * 4:(slot + 1) * 4, :],
                                          (hi if which == 0 else lo)[:])

        # ------------------------------------------------------------------
        # Phases A (scores + top-k) and B (gather+add, transpose, BN1 stats)
        # ------------------------------------------------------------------
        with nc.named_scope("phaseAB"), \
             tc.tile_pool(name="scps", bufs=2, space="PSUM") as scps, \
             tc.tile_pool(name="score", bufs=3) as scorep, \
             tc.tile_pool(name="idxp", bufs=6) as idxp, \
             tc.tile_pool(name="gat", bufs=6) as gatp, \
             tc.tile_pool(name="trps", bufs=3, space="PSUM") as trps, \
             tc.tile_pool(name="dmy", bufs=2) as dmyp:
            def emit_scores(m):
                # per-row packing params: beta = (S0/C)*exp(sqneg/3),
                # bias = BIAS0 + MAGIC + sqneg*beta  (score includes +|p_i|^2)
                sqn = idxp.tile([128, 1], f32, tag="sqn")
                nc.sync.dma_start(sqn[:], sqd.ap()[m * 128:(m + 1) * 128, :])
                beta = idxp.tile([128, 1], f32, tag="beta")
                nc.scalar.activation(beta[:], sqn[:], ACTF.Exp,
                                     scale=1.0 / 3.0)
                nc.vector.tensor_scalar_mul(beta[:], beta[:], S0 / CCLAMP)
                biasm = idxp.tile([128, 1], f32, tag="biasm")
                nc.vector.tensor_mul(biasm[:], sqn[:], beta[:])
                nc.vector.tensor_scalar_add(biasm[:], biasm[:], BIAS0 + MAGIC)
                score = scorep.tile([128, N], f32, tag="score")
                for j in range(N // 1024):
                    ps = scps.tile([128, 1024], f32, space="PSUM", tag="sc")
                    for h in range(2):
                        nc.tensor.matmul(
                            ps[:, h * 512:(h + 1) * 512],
                            lhsT=lhs_cat[:, bass.ts(m, 128)],
                            rhs=rhs_cat[:, j * 1024 + h * 512:
                                        j * 1024 + (h + 1) * 512],
                            start=True, stop=True)
                    # affine + integer-quantize (fp32 rounding at exp 23)
                    nc.scalar.activation(score[:, bass.ts(j, 1024)], ps[:],
                                         ACTF.Prelu, bias=biasm[:, 0:1],
                                         scale=beta[:, 0:1], alpha=1.0)
                # pack: score = (quantized - MAGIC) + j/4096
                for j in range(N // 1024):
                    nc.vector.scalar_tensor_tensor(
                        out=score[:, bass.ts(j, 1024)],
                        in0=score[:, bass.ts(j, 1024)], scalar=MAGIC,
                        in1=jv_sb[:, bass.ts(j, 1024)],
                        op0=mybir.AluOpType.subtract,
                        op1=mybir.AluOpType.add)
                return score

            pending = [emit_scores(0), emit_scores(1)]

            for m in range(NM):
                score = pending.pop(0) if pending else emit_scores(m)

                # L1: top-8 of each 256-col chunk (candidate values carry
                # their global column index in the low mantissa bits)
                cand = idxp.tile([128, 128], f32, tag="cand")
                for c in range(16):
                    nc.vector.max(out=cand[:, c * 8:(c + 1) * 8],
                                  in_=score[:, c * 256:(c + 1) * 256])
                # L2: top-24 of the candidates; col 0 is the self point
                sel = idxp.tile([128, 24], f32, tag="sel")
                dec = idxp.tile([128, 24], u32, tag="dec")
                # gather 8B neighbor coords (slot 0 = self, plain strided DMA)
                nbrP = gatp.tile([128, 128], f16, tag="nbrP")
                nc.sync.dma_start(nbrP[:, 0:4],
                                  pts16.ap()[m * 128:(m + 1) * 128, :])
                for r in range(3):
                    nc.vector.max(out=sel[:, r * 8:(r + 1) * 8], in_=cand[:])
                    if r < 2:
                        nc.vector.match_replace(
                            out=cand[:], in_to_replace=sel[:, r * 8:(r + 1) * 8],
                            in_values=cand[:], imm_value=NEG)
                    # decode this round's indices and issue its gathers so
                    # the Pool engine starts while later rounds still run
                    nc.vector.tensor_scalar(
                        out=dec[:, r * 8:(r + 1) * 8].bitcast(i32),
                        in0=sel[:, r * 8:(r + 1) * 8].bitcast(i32),
                        scalar1=4095, scalar2=None,
                        op0=mybir.AluOpType.bitwise_and)
                    for kk in range(max(1, r * 8), min(KNN, (r + 1) * 8)):
                        nc.gpsimd.indirect_dma_start(
                            out=nbrP[:, kk * 4:(kk + 1) * 4], out_offset=None,
                            in_=pts16.ap(),
                            in_offset=bass.IndirectOffsetOnAxis(
                                ap=dec[:, kk:kk + 1], axis=0))
                # emit the next score tile BEFORE the Pool-dependent
                # transpose/pair-matmuls: keeps the PE/ACT/DVE streams for
                # tile m+2 from queueing behind work that waits on this
                # tile's gathers (in-order engine queues)
                if m + 2 < NM:
                    pending.append(emit_scores(m + 2))
                nbrPT = gatp.tile([128, 128], f16, tag="nbrPT")
                nc.sync.dma_start_transpose(nbrPT[:], nbrP[:])
                # h1_pre pair blocks via K=8 (+K=4 self/C term) matmuls:
                # rows 0-63 = W1a@p_even + C, rows 64-127 = W1a@p_odd + C
                for gi, (j2a, j2b) in enumerate(PGROUPS):
                    gw = (j2b - j2a) * 128
                    trp = trps.tile([128, 512], f32, space="PSUM", tag="tr")
                    for g in range(j2b - j2a):
                        jp = j2a + g
                        nc.tensor.matmul(
                            trp[:, g * 128:(g + 1) * 128],
                            lhsT=w1g_sb[:, jp * 128:(jp + 1) * 128],
                            rhs=nbrPT[0:80, :],
                            start=True, stop=True)
                    col = (m * NJ2 + j2a) * 128
                    scol = m * 3 + gi
                    nc.scalar.activation(h1p[:, col:col + gw], trp[:, 0:gw],
                                         ACTF.Copy,
                                         accum_out=s1sum[:, scol:scol + 1])
                    dmy = dmyp.tile([128, 512], f32, tag="dmy")
                    nc.scalar.activation(dmy[:, 0:gw], trp[:, 0:gw],
                                         ACTF.Square,
                                         accum_out=s1sq[:, scol:scol + 1])

        # ------------------------------------------------------------------
        # BN1: global stats -> a1, b1
        # ------------------------------------------------------------------
        with nc.named_scope("bn1"), tc.tile_pool(name="bn1p", bufs=1) as bnp:
            st1 = bnp.tile([128, 2], f32)
            nc.vector.tensor_reduce(out=st1[:, 0:1], in_=s1sum[:],
                                    axis=mybir.AxisListType.X,
                                    op=mybir.AluOpType.add)
            nc.vector.tensor_reduce(out=st1[:, 1:2], in_=s1sq[:],
                                    axis=mybir.AxisListType.X,
                                    op=mybir.AluOpType.add)
            nc.sync.dma_start(cc1_in.ap(), st1[:])
            nc.gpsimd.collective_compute(
                kind="AllReduce", op=mybir.AluOpType.add,
                replica_groups=groups, ins=[cc1_in.ap()], outs=[cc1_out.ap()])
            st1g = bnp.tile([128, 2], f32)
            nc.sync.dma_start(st1g[:], cc1_out.ap())
            st1hi = bnp.tile([CH, 2], f32)
            nc.sync.dma_start(st1hi[:], st1g[CH:128, :])
            tot1 = bnp.tile([CH, 2], f32)
            nc.vector.tensor_add(tot1[:], st1g[0:CH, :], st1hi[:])
            mex = bnp.tile([CH, 2], f32)
            nc.scalar.mul(mex[:], tot1[:], 1.0 / CNT)
            mean1 = mex[:, 0:1]
            msq = bnp.tile([CH, 1], f32)
            nc.scalar.square(msq[:], mean1)
            var1 = bnp.tile([CH, 1], f32)
            nc.vector.tensor_sub(var1[:], mex[:, 1:2], msq[:])
            nc.scalar.activation(var1[:], var1[:], ACTF.Copy, bias=EPS)
            rcp1 = bnp.tile([CH, 1], f32)
            nc.vector.reciprocal(rcp1[:], var1[:])
            rs1 = bnp.tile([CH, 1], f32)
            nc.scalar.sqrt(rs1[:], rcp1[:])
            a1h = bnp.tile([CH, 1], f32)
            nc.vector.tensor_mul(a1h[:], rs1[:], g1_sb[:])
            am = bnp.tile([CH, 1], f32)
            nc.vector.tensor_mul(am[:], a1h[:], mean1)
            b1h = bnp.tile([CH, 1], f32)
            nc.vector.tensor_sub(b1h[:], b1in_sb[:], am[:])
            nc.sync.dma_start(a1_sb[0:CH, :], a1h[:])
            nc.sync.dma_start(a1_sb[CH:128, :], a1h[:])
            nc.sync.dma_start(b1_sb[0:CH, :], b1h[:])
            nc.sync.dma_start(b1_sb[CH:128, :], b1h[:])

        # ------------------------------------------------------------------
        # Phase C: h2_pre = W2 @ lrelu(a1*h1_pre + b1); BN2 stats (bn_stats)
        # ------------------------------------------------------------------
        cdpool = tc.alloc_tile_pool(name="cd", bufs=1)
        pooled_all = cdpool.tile([CH, N], f32)
        with nc.named_scope("phaseC"), \
             tc.tile_pool(name="h1a", bufs=2) as h1ap, \
             tc.tile_pool(name="pmx", bufs=4) as pmxp, \
             tc.tile_pool(name="h2ps", bufs=2, space="PSUM") as h2ps:
            for m in range(NM):
                mcol = m * NJ2 * 128
                h1a = h1ap.tile([128, NJ2 * 128], f16, tag="h1a")
                nc.scalar.activation(h1a[:], h1p[:, mcol:mcol + NJ2 * 128],
                                     ACTF.Prelu, bias=b1_sb[:, 0:1],
                                     scale=a1_sb[:, 0:1], alpha=ALPHA)
                # per-parity [64,1280] PSUM region; BN2 sums ride the
                # scalar-engine psum->sbuf copy (Copy/Square accum_out),
                # freeing the vector engine of bn_stats entirely
                pmaxes = []
                for par in range(2):
                    hp = h2ps.tile([CH, 3 * 512], f32, space="PSUM", tag="h2")
                    for c0, c1 in ((0, 512), (512, 1024), (1024, 1280)):
                        nc.tensor.matmul(
                            hp[:, c0:c1],
                            lhsT=w2t_sb[par * CH:(par + 1) * CH, :],
                            rhs=h1a[par * CH:(par + 1) * CH, c0:c1],
                            start=True, stop=True)
                    slot = m * 2 + par
                    hsb = pmxp.tile([CH, NJ2 * 128], f16, tag="hsb")
                    nc.scalar.activation(hsb[:], hp[:, 0:1280], ACTF.Copy,
                                         accum_out=s2c[:, slot:slot + 1])
                    dmy2 = pmxp.tile([CH, NJ2 * 128], f16, tag="dmy2")
                    nc.scalar.activation(dmy2[:], hp[:, 0:1280], ACTF.Square,
                                         accum_out=s2q[:, slot:slot + 1])
                    pm = pmxp.tile([CH, 128], f32, tag="pm")
                    hb = hsb[:]
                    rm_in = bass.AP(hb.tensor, hb.offset,
                                    [list(hb.ap[0]), [1, 128], [128, NJ2]])
                    nc.vector.tensor_reduce(
                        out=pm[:], in_=rm_in,
                        axis=mybir.AxisListType.X, op=mybir.AluOpType.max)
                    pmaxes.append(pm)
                nc.vector.tensor_tensor(
                    out=pooled_all[:, bass.ts(m, 128)], in0=pmaxes[0][:],
                    in1=pmaxes[1][:], op=mybir.AluOpType.max)

        # ------------------------------------------------------------------
        # BN2: aggregate + global stats -> a2, b2
        # ------------------------------------------------------------------
        with nc.named_scope("bn2"), tc.tile_pool(name="bn2p", bufs=1) as bnp:
            st2 = bnp.tile([CH, 2], f32)
            nc.vector.tensor_reduce(out=st2[:, 0:1], in_=s2c[:],
                                    axis=mybir.AxisListType.X,
                                    op=mybir.AluOpType.add)
            nc.vector.tensor_reduce(out=st2[:, 1:2], in_=s2q[:],
                                    axis=mybir.AxisListType.X,
                                    op=mybir.AluOpType.add)
            nc.scalar.mul(st2[:], st2[:], B / float(CNT))
            nc.sync.dma_start(cc2_in.ap(), st2[:])
            nc.gpsimd.collective_compute(
                kind="AllReduce", op=mybir.AluOpType.add,
                replica_groups=groups, ins=[cc2_in.ap()], outs=[cc2_out.ap()])
            tot2 = bnp.tile([CH, 2], f32)
            nc.sync.dma_start(tot2[:], cc2_out.ap())
            mean2 = bnp.tile([CH, 1], f32)
            nc.scalar.mul(mean2[:], tot2[:, 0:1], 1.0 / B)
            ex2b = bnp.tile([CH, 1], f32)
            nc.scalar.mul(ex2b[:], tot2[:, 1:2], 1.0 / B)
            msq2b = bnp.tile([CH, 1], f32)
            nc.scalar.square(msq2b[:], mean2[:])
            var2 = bnp.tile([CH, 1], f32)
            nc.vector.tensor_sub(var2[:], ex2b[:], msq2b[:])
            nc.scalar.activation(var2[:], var2[:], ACTF.Copy, bias=EPS)
            rcp2 = bnp.tile([CH, 1], f32)
            nc.vector.reciprocal(rcp2[:], var2[:])
            rs2 = bnp.tile([CH, 1], f32)
            nc.scalar.sqrt(rs2[:], rcp2[:])
            nc.vector.tensor_mul(a2_sb[:], rs2[:], g2_sb[:])
            am2 = bnp.tile([CH, 1], f32)
            nc.vector.tensor_mul(am2[:], a2_sb[:], mean2[:])
            nc.vector.tensor_sub(b2_sb[:], b2in_sb[:], am2[:])

        # ------------------------------------------------------------------
        # Phase D: recompute h2, apply BN2 + lrelu, max-pool over neighbors
        # ------------------------------------------------------------------
        with nc.named_scope("phaseD"), \
             tc.tile_pool(name="runm", bufs=4) as runp:
            for m in range(NM):
                runmax = runp.tile([CH, 128], f32, tag="run")
                nc.scalar.activation(runmax[:], pooled_all[:, bass.ts(m, 128)],
                                     ACTF.Prelu, bias=b2_sb[:, 0:1],
                                     scale=a2_sb[:, 0:1], alpha=ALPHA)
                nc.sync.dma_start(out_t.ap()[:, bass.ts(m, 128)], runmax[:])

        cdpool.release()
        abpool.release()
        const.release()
        dramp.release()

    return nc


_prog_cache = {}


def _get_program():
    if "nc" not in _prog_cache:
        _prog_cache["nc"] = _build_program()
    return _prog_cache["nc"]


def make_in_maps(x, W1, gamma1, beta1, W2, gamma2, beta2):
    x = np.asarray(x, dtype=np.float32)
    W1 = np.asarray(W1, dtype=np.float32)
    W2 = np.asarray(W2, dtype=np.float32)
    W1aT = W1[:, 0:3].T                      # [3, 64]
    W1cT = (W1[:, 3:6] - W1[:, 0:3]).T
    w1g = np.zeros((80, 1280), np.float32)
    for jp in range(10):
        blk = w1g[:, jp * 128:(jp + 1) * 128]
        blk[8 * jp:8 * jp + 3, 0:64] += W1aT
        blk[8 * jp + 4:8 * jp + 7, 64:128] += W1aT
        blk[0:3, 0:64] += W1cT
        blk[0:3, 64:128] += W1cT
    w1g = w1g.astype(np.float16)
    g2 = np.asarray(gamma2, dtype=np.float32).reshape(CH)
    sgn2 = np.where(g2 < 0, -1.0, 1.0).astype(np.float32)
    W2f = W2 * sgn2[:, None]          # flip rows so the BN2 scale is >= 0
    w2t_1 = np.ascontiguousarray(W2f.T).astype(np.float16)    # [64, 64]
    w2t = np.concatenate([w2t_1, w2t_1], axis=0)              # [128, 64]
    col = lambda v: np.ascontiguousarray(
        np.asarray(v, dtype=np.float32).reshape(CH, 1))
    jv = np.ascontiguousarray(np.broadcast_to(
        (np.arange(N, dtype=np.float32) / 4096.0), (128, N))).astype(np.float32)
    return [{
        "xb": np.concatenate([x[b], np.ones((1, N), np.float32)], axis=0),
        "pts16": np.concatenate([x[b].T.astype(np.float16),
                                 np.ones((N, 1), np.float16)], axis=1),
        "jv": jv,
        "w1g": w1g, "w2t": w2t,
        "bn1g": col(gamma1), "bn1b": col(beta1),
        "bn2g": col(np.abs(g2)), "bn2b": col(beta2),
    } for b in range(B)]


def kernel(x, W1, gamma1, beta1, W2, gamma2, beta2):
    nc = _get_program()
    in_maps = make_in_maps(x, W1, gamma1, beta1, W2, gamma2, beta2)
    res = run_bass_kernel_spmd(nc, in_maps, list(range(B)))
    out = np.stack([res.results[b]["out"] for b in range(B)], axis=0)
    return out.astype(np.float32)



# revision 42
# speedup vs baseline: 1.0583x; 1.0583x over previous
"""Trainium2 Bass kernel for a DGCNN-style point-cloud encoder.

Per batch element (one per NeuronCore, B=8): kNN graph (k=20) over N=4096
points via a distance matmul + iterative top-8 extraction (max/max_index/
match_replace), edge-feature MLP with two training-mode batchnorms (global
stats via cross-core AllReduce) and leaky-relu, then max-pool over
neighbors.  Layout strategy: the first MLP layer is decomposed into
per-point projections A = W1a@p and C = (W1b-W1a)@p; the gather of A rows
by neighbor index runs as an indirect DMA with compute_op=add onto a
C-prefilled tile, so h1_pre arrives in one pass; PE transposes pairs of
neighbors into channelx2 PSUM tiles for the W2 stage.
"""
import sys
sys.path.insert(0, '/opt/trn_rl_repo')

import numpy as np
import orjson

import concourse.bass as bass
import concourse.mybir as mybir
import concourse.tile as tile
from concourse import library_config
from concourse.bass_utils import run_bass_kernel_spmd

# ---------------------------------------------------------------------------
# Workaround for walrus 'Too many sync wait commands': this toolchain accepts
# at most one sem-wait per lowered instruction. Split any instruction carrying
# more waits into EventSemaphore wait-carriers placed immediately before it.
# ---------------------------------------------------------------------------
_MAXW = 1


def _split_excess_waits(j) -> bool:
    changed = False
    for fn in j.get("functions", []):
        for blk in fn.get("blocks", []):
            out = []
            for inst in blk.get("instructions", []):
                si = inst.get("sync_info") or {}
                ow = si.get("on_wait") or []
                if len(ow) > _MAXW:
                    changed = True
                    chunks = [ow[i:i + _MAXW] for i in range(0, len(ow), _MAXW)]
                    for ci, chunk in enumerate(chunks[:-1]):
                        out.append({
                            "debug": inst.get("debug", 0),
                            "engine": inst["engine"],
                            "ins": [], "outs": [],
                            "name": f"{inst['name']}-w{ci}",
                            "opcode": "EventSemaphore",
                            "sync_info": {"on_update": [], "on_wait": chunk},
                        })
                    si = dict(si)
                    si["on_wait"] = chunks[-1]
                    inst = dict(inst)
                    inst["sync_info"] = si
                out.append(inst)
            blk["instructions"] = out
    return changed


_orig_to_json_bytes = bass.Bass.to_json_bytes


def _patched_to_json_bytes(self) -> bytes:
    raw = _orig_to_json_bytes(self)
    j = orjson.loads(raw)
    if _split_excess_waits(j):
        return orjson.dumps(j)
    return raw


bass.Bass.to_json_bytes = _patched_to_json_bytes

# ---------------------------------------------------------------------------
# Problem constants (hardcoded; kernel.py must be self-contained)
# ---------------------------------------------------------------------------
B = 8            # batch = number of cores
N = 4096         # points per cloud
KNN = 20         # neighbors
CH = 64          # hidden channels
EPS = 1e-5
ALPHA = 0.2      # leaky-relu slope
NM = N // 128    # 32 row-tiles
NJ2 = KNN // 2   # 10 neighbor pairs
CNT = B * N * KNN  # batchnorm population size (global over all cores)
NEG = -1.0e30
PGROUPS = [(0, 4), (4, 8), (8, 10)]     # j2 pair-groups per psum tile
WCHUNKS = [(0, 512), (512, 1024), (1024, 1280)]  # W2 rhs chunks per parity

# --- packed top-k constants -------------------------------------------------
# Scores are quantized per-row to an 11-bit field with the 12-bit global
# column index packed into the low mantissa bits: after the scalar-engine
# affine (score*beta_row + bias_row + MAGIC), fp32 rounding at exponent 23
# quantizes to integers; subtracting MAGIC and adding j/4096 yields
# packed = q + j/4096 in [2048, 4096) whose low 12 mantissa bits are j.
# beta_row = (S0/CCLAMP)*exp(-|p_i|^2/3) adapts the clamp window to the
# local point density (validated: max_row d24^2/exp(sq/3) = 0.157 < 0.22).
MAGIC = 12582912.0          # 1.5 * 2^23
BIAS0 = 4050.0
S0 = 2002.0
CCLAMP = 0.22
LNB0 = 9.116029692504942    # ln(S0 / CCLAMP)

f32 = mybir.dt.float32
f16 = mybir.dt.float16
u32 = mybir.dt.uint32
u16 = mybir.dt.uint16
i16 = mybir.dt.int16
i32 = mybir.dt.int32
ACTF = mybir.ActivationFunctionType


def _bcast_mid(ap, reps):
    """Insert a step-0 dim after the partition dim: [P, F] -> [P, reps, F]."""
    return bass.AP(ap.tensor, ap.offset,
                   [list(ap.ap[0]), [0, reps], list(ap.ap[1])])


def _build_program():
    nc = bass.Bass("TRN2", target_bir_lowering=False, debug=False,
                   num_devices=B)

    xb = nc.dram_tensor("xb", [4, N], f32, kind="ExternalInput")
    jvt = nc.dram_tensor("jv", [128, N], f32, kind="ExternalInput")
    sqd = nc.dram_tensor("sqd", [N, 1], f32)
    w1g = nc.dram_tensor("w1g", [80, NJ2 * 128], f16, kind="ExternalInput")
    pts16 = nc.dram_tensor("pts16", [N, 4], f16, kind="ExternalInput")
    w2t = nc.dram_tensor("w2t", [128, CH], f16, kind="ExternalInput")
    bn1g = nc.dram_tensor("bn1g", [CH, 1], f32, kind="ExternalInput")
    bn1b = nc.dram_tensor("bn1b", [CH, 1], f32, kind="ExternalInput")
    bn2g = nc.dram_tensor("bn2g", [CH, 1], f32, kind="ExternalInput")
    bn2b = nc.dram_tensor("bn2b", [CH, 1], f32, kind="ExternalInput")
    out_t = nc.dram_tensor("out", [CH, N], f32, kind="ExternalOutput")

    cc1_in = nc.dram_tensor("cc1_in", [128, 2], f32)
    cc1_out = nc.dram_tensor("cc1_out", [128, 2], f32, addr_space="Shared")
    cc2_in = nc.dram_tensor("cc2_in", [CH, 2], f32)
    cc2_out = nc.dram_tensor("cc2_out", [CH, 2], f32, addr_space="Shared")
    groups = [list(range(B))]

    with tile.TileContext(nc) as tc:
        const = tc.alloc_tile_pool(name="const", bufs=1)
        dramp = tc.alloc_tile_pool(name="dram", bufs=1, space="DRAM")
        abpool = tc.alloc_tile_pool(name="ab", bufs=1)

        # whole-kernel tensors
        w2t_sb = const.tile([128, CH], f16)
        g1_sb = const.tile([CH, 1], f32)
        b1in_sb = const.tile([CH, 1], f32)
        g2_sb = const.tile([CH, 1], f32)
        b2in_sb = const.tile([CH, 1], f32)
        h1p = const.tile([128, NM * NJ2 * 128], f16)   # stored h1_pre
        a1_sb = const.tile([128, 1], f32)
        b1_sb = const.tile([128, 1], f32)
        a2_sb = const.tile([CH, 1], f32)
        b2_sb = const.tile([CH, 1], f32)
        s2h = const.tile([128, NM], f32)               # phaseC sum(h1a) slots
        s2q = const.tile([CH, NM * 2], f32)            # phaseC sum-sq slots
        jv_sb = const.tile([128, N], f32)              # j/4096 index column vec

        # phase A/B tensors (released after phase AB)
        # K=12 concatenated split operands: [hi;hi;lo] x [hi;lo;hi] computes
        # hi*hi + hi*lo + lo*hi in ONE matmul (fp32 to ~2^-24 rel)
        lhs_cat = abpool.tile([12, N], f16)
        rhs_cat = abpool.tile([12, N], f16)
        w1g_sb = abpool.tile([80, NJ2 * 128], f16)
        s1sum = abpool.tile([128, 3 * NM], f32)
        s1sq = abpool.tile([128, 3 * NM], f32)

        nc.sync.dma_start(w1g_sb[:], w1g.ap())
        nc.sync.dma_start(w2t_sb[:], w2t.ap())
        nc.sync.dma_start(g1_sb[:], bn1g.ap())
        nc.sync.dma_start(b1in_sb[:], bn1b.ap())
        nc.sync.dma_start(g2_sb[:], bn2g.ap())
        nc.sync.dma_start(b2in_sb[:], bn2b.ap())
        nc.sync.dma_start(jv_sb[:], jvt.ap())

        with nc.named_scope("stage0"):
            with tc.tile_pool(name="s0", bufs=1) as s0big, \
                 tc.tile_pool(name="s0s", bufs=1) as s0pool, \
                 tc.tile_pool(name="s0ps", bufs=2, space="PSUM") as s0psum:
                lhs_all = s0big.tile([4, N], f32, tag="lhs")
                rhs_all = s0big.tile([4, N], f32, tag="rhs")
                nc.sync.dma_start(lhs_all[:], xb.ap())
                nc.scalar.mul(rhs_all[0:3, :], lhs_all[0:3, :], 2.0)
                xsq = s0big.tile([4, N], f32, tag="scratch")
                nc.scalar.square(xsq[0:3, :], lhs_all[0:3, :])
                ones3 = nc.const_aps.tensor(1.0, (3, 1), f32)
                for j in range(N // 512):
                    ps = s0psum.tile([1, 512], f32, space="PSUM", tag="sq")
                    nc.tensor.matmul(ps[:], lhsT=ones3,
                                     rhs=xsq[0:3, bass.ts(j, 512)],
                                     start=True, stop=True)
                    sqneg = s0pool.tile([1, 512], f32, tag="sqneg")
                    nc.scalar.activation(sqneg[:], ps[:], ACTF.Copy, scale=-1.0)
                    nc.sync.dma_start(rhs_all[3:4, bass.ts(j, 512)], sqneg[:])
                nc.sync.dma_start(sqd.ap(), rhs_all[3:4, :])
                # fp16 hi/lo split of lhs/rhs for single-pass PE matmuls
                for full, cat, pattern in ((lhs_all, lhs_cat, (0, 0, 1)),
                                           (rhs_all, rhs_cat, (0, 1, 0))):
                    hi = s0pool.tile([4, N], f16, tag="hi")
                    nc.scalar.copy(hi[:], full[:])
                    rb = s0big.tile([4, N], f32, tag="scratch")
                    nc.vector.tensor_sub(rb[:], full[:], hi[:])
                    lo = s0pool.tile([4, N], f16, tag="lo")
                    nc.scalar.copy(lo[:], rb[:])
                    for slot, which in enumerate(pattern):
                        nc.sync.dma_start(cat[slot * 4:(slot + 1) * 4, :],
                                          (hi if which == 0 else lo)[:])

        # ------------------------------------------------------------------
        # Phases A (scores + top-k) and B (gather+add, transpose, BN1 stats)
        # ------------------------------------------------------------------
        with nc.named_scope("phaseAB"), \
             tc.tile_pool(name="scps", bufs=2, space="PSUM") as scps, \
             tc.tile_pool(name="score", bufs=3) as scorep, \
             tc.tile_pool(name="idxp", bufs=6) as idxp, \
             tc.tile_pool(name="gat", bufs=6) as gatp, \
             tc.tile_pool(name="trps", bufs=3, space="PSUM") as trps, \
             tc.tile_pool(name="dmy", bufs=2) as dmyp:
            def emit_scores(m):
                # per-row packing params: beta = (S0/C)*exp(sqneg/3),
                # bias = BIAS0 + MAGIC + sqneg*beta  (score includes +|p_i|^2)
                sqn = idxp.tile([128, 1], f32, tag="sqn")
                nc.sync.dma_start(sqn[:], sqd.ap()[m * 128:(m + 1) * 128, :])
                beta = idxp.tile([128, 1], f32, tag="beta")
                nc.scalar.activation(beta[:], sqn[:], ACTF.Exp,
                                     scale=1.0 / 3.0)
                nc.vector.tensor_scalar_mul(beta[:], beta[:], S0 / CCLAMP)
                biasm = idxp.tile([128, 1], f32, tag="biasm")
                nc.vector.tensor_mul(biasm[:], sqn[:], beta[:])
                nc.vector.tensor_scalar_add(biasm[:], biasm[:], BIAS0 + MAGIC)
                score = scorep.tile([128, N], f32, tag="score")
                for j in range(N // 1024):
                    ps = scps.tile([128, 1024], f32, space="PSUM", tag="sc")
                    for h in range(2):
                        nc.tensor.matmul(
                            ps[:, h * 512:(h + 1) * 512],
                            lhsT=lhs_cat[:, bass.ts(m, 128)],
                            rhs=rhs_cat[:, j * 1024 + h * 512:
                                        j * 1024 + (h + 1) * 512],
                            start=True, stop=True)
                    # affine + integer-quantize (fp32 rounding at exp 23)
                    nc.scalar.activation(score[:, bass.ts(j, 1024)], ps[:],
                                         ACTF.Prelu, bias=biasm[:, 0:1],
                                         scale=beta[:, 0:1], alpha=1.0)
                # pack: score = (quantized - MAGIC) + j/4096
                for j in range(N // 1024):
                    nc.vector.scalar_tensor_tensor(
                        out=score[:, bass.ts(j, 1024)],
                        in0=score[:, bass.ts(j, 1024)], scalar=MAGIC,
                        in1=jv_sb[:, bass.ts(j, 1024)],
                        op0=mybir.AluOpType.subtract,
                        op1=mybir.AluOpType.add)
                return score

            pending = [emit_scores(0), emit_scores(1)]

            for m in range(NM):
                score = pending.pop(0) if pending else emit_scores(m)

                # L1: top-8 of each 256-col chunk (candidate values carry
                # their global column index in the low mantissa bits)
                cand = idxp.tile([128, 128], f32, tag="cand")
                for c in range(16):
                    nc.vector.max(out=cand[:, c * 8:(c + 1) * 8],
                                  in_=score[:, c * 256:(c + 1) * 256])
                # L2: top-24 of the candidates; col 0 is the self point
                sel = idxp.tile([128, 24], f32, tag="sel")
                dec = idxp.tile([128, 24], u32, tag="dec")
                # gather 8B neighbor coords (slot 0 = self, plain strided DMA)
                nbrP = gatp.tile([128, 128], f16, tag="nbrP")
                nc.sync.dma_start(nbrP[:, 0:4],
                                  pts16.ap()[m * 128:(m + 1) * 128, :])
                for r in range(3):
                    nc.vector.max(out=sel[:, r * 8:(r + 1) * 8], in_=cand[:])
                    if r < 2:
                        nc.vector.match_replace(
                            out=cand[:], in_to_replace=sel[:, r * 8:(r + 1) * 8],
                            in_values=cand[:], imm_value=NEG)
                    # decode this round's indices and issue its gathers so
                    # the Pool engine starts while later rounds still run
                    nc.vector.tensor_scalar(
                        out=dec[:, r * 8:(r + 1) * 8].bitcast(i32),
                        in0=sel[:, r * 8:(r + 1) * 8].bitcast(i32),
                        scalar1=4095, scalar2=None,
                        op0=mybir.AluOpType.bitwise_and)
                    for kk in range(max(1, r * 8), min(KNN, (r + 1) * 8)):
                        nc.gpsimd.indirect_dma_start(
                            out=nbrP[:, kk * 4:(kk + 1) * 4], out_offset=None,
                            in_=pts16.ap(),
                            in_offset=bass.IndirectOffsetOnAxis(
                                ap=dec[:, kk:kk + 1], axis=0))
                # emit the next score tile BEFORE the Pool-dependent
                # transpose/pair-matmuls: keeps the PE/ACT/DVE streams for
                # tile m+2 from queueing behind work that waits on this
                # tile's gathers (in-order engine queues)
                if m + 2 < NM:
                    pending.append(emit_scores(m + 2))
                nbrPT = gatp.tile([128, 128], f16, tag="nbrPT")
                nc.sync.dma_start_transpose(nbrPT[:], nbrP[:])
                # h1_pre pair blocks via K=8 (+K=4 self/C term) matmuls:
                # rows 0-63 = W1a@p_even + C, rows 64-127 = W1a@p_odd + C
                for gi, (j2a, j2b) in enumerate(PGROUPS):
                    gw = (j2b - j2a) * 128
                    trp = trps.tile([128, 512], f32, space="PSUM", tag="tr")
                    for g in range(j2b - j2a):
                        jp = j2a + g
                        nc.tensor.matmul(
                            trp[:, g * 128:(g + 1) * 128],
                            lhsT=w1g_sb[:, jp * 128:(jp + 1) * 128],
                            rhs=nbrPT[0:80, :],
                            start=True, stop=True)
                    col = (m * NJ2 + j2a) * 128
                    scol = m * 3 + gi
                    nc.scalar.activation(h1p[:, col:col + gw], trp[:, 0:gw],
                                         ACTF.Copy,
                                         accum_out=s1sum[:, scol:scol + 1])
                    dmy = dmyp.tile([128, 512], f32, tag="dmy")
                    nc.scalar.activation(dmy[:, 0:gw], trp[:, 0:gw],
                                         ACTF.Square,
                                         accum_out=s1sq[:, scol:scol + 1])

        # ------------------------------------------------------------------
        # BN1: global stats -> a1, b1
        # ------------------------------------------------------------------
        with nc.named_scope("bn1"), tc.tile_pool(name="bn1p", bufs=1) as bnp:
            st1 = bnp.tile([128, 2], f32)
            nc.vector.tensor_reduce(out=st1[:, 0:1], in_=s1sum[:],
                                    axis=mybir.AxisListType.X,
                                    op=mybir.AluOpType.add)
            nc.vector.tensor_reduce(out=st1[:, 1:2], in_=s1sq[:],
                                    axis=mybir.AxisListType.X,
                                    op=mybir.AluOpType.add)
            nc.sync.dma_start(cc1_in.ap(), st1[:])
            nc.gpsimd.collective_compute(
                kind="AllReduce", op=mybir.AluOpType.add,
                replica_groups=groups, ins=[cc1_in.ap()], outs=[cc1_out.ap()])
            st1g = bnp.tile([128, 2], f32)
            nc.sync.dma_start(st1g[:], cc1_out.ap())
            st1hi = bnp.tile([CH, 2], f32)
            nc.sync.dma_start(st1hi[:], st1g[CH:128, :])
            tot1 = bnp.tile([CH, 2], f32)
            nc.vector.tensor_add(tot1[:], st1g[0:CH, :], st1hi[:])
            mex = bnp.tile([CH, 2], f32)
            nc.scalar.mul(mex[:], tot1[:], 1.0 / CNT)
            mean1 = mex[:, 0:1]
            msq = bnp.tile([CH, 1], f32)
            nc.scalar.square(msq[:], mean1)
            var1 = bnp.tile([CH, 1], f32)
            nc.vector.tensor_sub(var1[:], mex[:, 1:2], msq[:])
            nc.scalar.activation(var1[:], var1[:], ACTF.Copy, bias=EPS)
            rcp1 = bnp.tile([CH, 1], f32)
            nc.vector.reciprocal(rcp1[:], var1[:])
            rs1 = bnp.tile([CH, 1], f32)
            nc.scalar.sqrt(rs1[:], rcp1[:])
            a1h = bnp.tile([CH, 1], f32)
            nc.vector.tensor_mul(a1h[:], rs1[:], g1_sb[:])
            am = bnp.tile([CH, 1], f32)
            nc.vector.tensor_mul(am[:], a1h[:], mean1)
            b1h = bnp.tile([CH, 1], f32)
            nc.vector.tensor_sub(b1h[:], b1in_sb[:], am[:])
            nc.sync.dma_start(a1_sb[0:CH, :], a1h[:])
            nc.sync.dma_start(a1_sb[CH:128, :], a1h[:])
            nc.sync.dma_start(b1_sb[0:CH, :], b1h[:])
            nc.sync.dma_start(b1_sb[CH:128, :], b1h[:])

        # ------------------------------------------------------------------
        # Phase C: h2_pre = W2 @ lrelu(a1*h1_pre + b1); BN2 stats (bn_stats)
        # ------------------------------------------------------------------
        cdpool = tc.alloc_tile_pool(name="cd", bufs=1)
        pooled_all = cdpool.tile([CH, N], f32)
        with nc.named_scope("phaseC"), \
             tc.tile_pool(name="h1a", bufs=2) as h1ap, \
             tc.tile_pool(name="pmx", bufs=4) as pmxp, \
             tc.tile_pool(name="h2ps", bufs=2, space="PSUM") as h2ps:
            for m in range(NM):
                mcol = m * NJ2 * 128
                h1a = h1ap.tile([128, NJ2 * 128], f16, tag="h1a")
                nc.scalar.activation(h1a[:], h1p[:, mcol:mcol + NJ2 * 128],
                                     ACTF.Prelu, bias=b1_sb[:, 0:1],
                                     scale=a1_sb[:, 0:1], alpha=ALPHA,
                                     accum_out=s2h[:, m:m + 1])
                # per-parity [64,1280] PSUM region; BN2 sums ride the
                # scalar-engine psum->sbuf copy (Copy/Square accum_out),
                # freeing the vector engine of bn_stats entirely
                pmaxes = []
                for par in range(2):
                    hp = h2ps.tile([CH, 3 * 512], f32, space="PSUM", tag="h2")
                    for c0, c1 in ((0, 512), (512, 1024), (1024, 1280)):
                        nc.tensor.matmul(
                            hp[:, c0:c1],
                            lhsT=w2t_sb[par * CH:(par + 1) * CH, :],
                            rhs=h1a[par * CH:(par + 1) * CH, c0:c1],
                            start=True, stop=True)
                    slot = m * 2 + par
                    dmy2 = pmxp.tile([CH, NJ2 * 128], f16, tag="dmy2")
                    nc.scalar.activation(dmy2[:], hp[:, 0:1280], ACTF.Square,
                                         accum_out=s2q[:, slot:slot + 1])
                    pm = pmxp.tile([CH, 128], f32, tag="pm")
                    hb = hp[:]
                    rm_in = bass.AP(hb.tensor, hb.offset,
                                    [list(hb.ap[0]), [1, 128], [128, NJ2]])
                    nc.vector.tensor_reduce(
                        out=pm[:], in_=rm_in,
                        axis=mybir.AxisListType.X, op=mybir.AluOpType.max)
                    pmaxes.append(pm)
                nc.vector.tensor_tensor(
                    out=pooled_all[:, bass.ts(m, 128)], in0=pmaxes[0][:],
                    in1=pmaxes[1][:], op=mybir.AluOpType.max)

        # ------------------------------------------------------------------
        # BN2: aggregate + global stats -> a2, b2
        # ------------------------------------------------------------------
        with nc.named_scope("bn2"), tc.tile_pool(name="bn2p", bufs=1) as bnp, \
             tc.tile_pool(name="bn2ps", bufs=1, space="PSUM") as bn2ps:
            # sum(h2) via linearity: sum_cols h2 = W2f @ row-fold(sum h1a)
            sh = bnp.tile([128, 1], f32)
            nc.vector.tensor_reduce(out=sh[:], in_=s2h[:],
                                    axis=mybir.AxisListType.X,
                                    op=mybir.AluOpType.add)
            shh = bnp.tile([CH, 1], f32)
            nc.sync.dma_start(shh[:], sh[CH:128, :])
            mh = bnp.tile([CH, 1], f32)
            nc.vector.tensor_add(mh[:], sh[0:CH, :], shh[:])
            mh16 = bnp.tile([CH, 1], f16)
            nc.vector.tensor_copy(mh16[:], mh[:])
            sps = bn2ps.tile([CH, 1], f32, space="PSUM")
            nc.tensor.matmul(sps[:], lhsT=w2t_sb[0:CH, :], rhs=mh16[:],
                             start=True, stop=True)
            st2 = bnp.tile([CH, 2], f32)
            nc.scalar.activation(st2[:, 0:1], sps[:], ACTF.Copy,
                                 scale=B / float(CNT))
            nc.vector.tensor_reduce(out=st2[:, 1:2], in_=s2q[:],
                                    axis=mybir.AxisListType.X,
                                    op=mybir.AluOpType.add)
            nc.scalar.mul(st2[:, 1:2], st2[:, 1:2], B / float(CNT))
            nc.sync.dma_start(cc2_in.ap(), st2[:])
            nc.gpsimd.collective_compute(
                kind="AllReduce", op=mybir.AluOpType.add,
                replica_groups=groups, ins=[cc2_in.ap()], outs=[cc2_out.ap()])
            tot2 = bnp.tile([CH, 2], f32)
            nc.sync.dma_start(tot2[:], cc2_out.ap())
            mean2 = bnp.tile([CH, 1], f32)
            nc.scalar.mul(mean2[:], tot2[:, 0:1], 1.0 / B)
            ex2b = bnp.tile([CH, 1], f32)
            nc.scalar.mul(ex2b[:], tot2[:, 1:2], 1.0 / B)
            msq2b = bnp.tile([CH, 1], f32)
            nc.scalar.square(msq2b[:], mean2[:])
            var2 = bnp.tile([CH, 1], f32)
            nc.vector.tensor_sub(var2[:], ex2b[:], msq2b[:])
            nc.scalar.activation(var2[:], var2[:], ACTF.Copy, bias=EPS)
            rcp2 = bnp.tile([CH, 1], f32)
            nc.vector.reciprocal(rcp2[:], var2[:])
            rs2 = bnp.tile([CH, 1], f32)
            nc.scalar.sqrt(rs2[:], rcp2[:])
            nc.vector.tensor_mul(a2_sb[:], rs2[:], g2_sb[:])
            am2 = bnp.tile([CH, 1], f32)
            nc.vector.tensor_mul(am2[:], a2_sb[:], mean2[:])
            nc.vector.tensor_sub(b2_sb[:], b2in_sb[:], am2[:])

        # ------------------------------------------------------------------
        # Phase D: recompute h2, apply BN2 + lrelu, max-pool over neighbors
        # ------------------------------------------------------------------
        with nc.named_scope("phaseD"), \
             tc.tile_pool(name="runm", bufs=4) as runp:
            for m in range(NM):
                runmax = runp.tile([CH, 128], f32, tag="run")
                nc.scalar.activation(runmax[:], pooled_all[:, bass.ts(m, 128)],
                                     ACTF.Prelu, bias=b2_sb[:, 0:1],
                                     scale=a2_sb[:, 0:1], alpha=ALPHA)
                nc.sync.dma_start(out_t.ap()[:, bass.ts(m, 128)], runmax[:])

        cdpool.release()
        abpool.release()
        const.release()
        dramp.release()

    return nc


_prog_cache = {}


def _get_program():
    if "nc" not in _prog_cache:
        _prog_cache["nc"] = _build_program()
    return _prog_cache["nc"]


def make_in_maps(x, W1, gamma1, beta1, W2, gamma2, beta2):
    x = np.asarray(x, dtype=np.float32)
    W1 = np.asarray(W1, dtype=np.float32)
    W2 = np.asarray(W2, dtype=np.float32)
    W1aT = W1[:, 0:3].T                      # [3, 64]
    W1cT = (W1[:, 3:6] - W1[:, 0:3]).T
    w1g = np.zeros((80, 1280), np.float32)
    for jp in range(10):
        blk = w1g[:, jp * 128:(jp + 1) * 128]
        blk[8 * jp:8 * jp + 3, 0:64] += W1aT
        blk[8 * jp + 4:8 * jp + 7, 64:128] += W1aT
        blk[0:3, 0:64] += W1cT
        blk[0:3, 64:128] += W1cT
    w1g = w1g.astype(np.float16)
    g2 = np.asarray(gamma2, dtype=np.float32).reshape(CH)
    sgn2 = np.where(g2 < 0, -1.0, 1.0).astype(np.float32)
    W2f = W2 * sgn2[:, None]          # flip rows so the BN2 scale is >= 0
    w2t_1 = np.ascontiguousarray(W2f.T).astype(np.float16)    # [64, 64]
    w2t = np.concatenate([w2t_1, w2t_1], axis=0)              # [128, 64]
    col = lambda v: np.ascontiguousarray(
        np.asarray(v, dtype=np.float32).reshape(CH, 1))
    jv = np.ascontiguousarray(np.broadcast_to(
        (np.arange(N, dtype=np.float32) / 4096.0), (128, N))).astype(np.float32)
    return [{
        "xb": np.concatenate([x[b], np.ones((1, N), np.float32)], axis=0),
        "pts16": np.concatenate([x[b].T.astype(np.float16),
                                 np.ones((N, 1), np.float16)], axis=1),
        "jv": jv,
        "w1g": w1g, "w2t": w2t,
        "bn1g": col(gamma1), "bn1b": col(beta1),
        "bn2g": col(np.abs(g2)), "bn2b": col(beta2),
    } for b in range(B)]


def kernel(x, W1, gamma1, beta1, W2, gamma2, beta2):
    nc = _get_program()
    in_maps = make_in_maps(x, W1, gamma1, beta1, W2, gamma2, beta2)
    res = run_bass_kernel_spmd(nc, in_maps, list(range(B)))
    out = np.stack([res.results[b]["out"] for b in range(B)], axis=0)
    return out.astype(np.float32)



# revision 43
# speedup vs baseline: 1.1100x; 1.0488x over previous
"""Trainium2 Bass kernel for a DGCNN-style point-cloud encoder.

Per batch element (one per NeuronCore, B=8): kNN graph (k=20) over N=4096
points via a distance matmul + iterative top-8 extraction (max/max_index/
match_replace), edge-feature MLP with two training-mode batchnorms (global
stats via cross-core AllReduce) and leaky-relu, then max-pool over
neighbors.  Layout strategy: the first MLP layer is decomposed into
per-point projections A = W1a@p and C = (W1b-W1a)@p; the gather of A rows
by neighbor index runs as an indirect DMA with compute_op=add onto a
C-prefilled tile, so h1_pre arrives in one pass; PE transposes pairs of
neighbors into channelx2 PSUM tiles for the W2 stage.
"""
import sys
sys.path.insert(0, '/opt/trn_rl_repo')

import numpy as np
import orjson

import concourse.bass as bass
import concourse.mybir as mybir
import concourse.tile as tile
from concourse import library_config
from concourse.bass_utils import run_bass_kernel_spmd

# ---------------------------------------------------------------------------
# Workaround for walrus 'Too many sync wait commands': this toolchain accepts
# at most one sem-wait per lowered instruction. Split any instruction carrying
# more waits into EventSemaphore wait-carriers placed immediately before it.
# ---------------------------------------------------------------------------
_MAXW = 1


def _split_excess_waits(j) -> bool:
    changed = False
    for fn in j.get("functions", []):
        for blk in fn.get("blocks", []):
            out = []
            for inst in blk.get("instructions", []):
                si = inst.get("sync_info") or {}
                ow = si.get("on_wait") or []
                if len(ow) > _MAXW:
                    changed = True
                    chunks = [ow[i:i + _MAXW] for i in range(0, len(ow), _MAXW)]
                    for ci, chunk in enumerate(chunks[:-1]):
                        out.append({
                            "debug": inst.get("debug", 0),
                            "engine": inst["engine"],
                            "ins": [], "outs": [],
                            "name": f"{inst['name']}-w{ci}",
                            "opcode": "EventSemaphore",
                            "sync_info": {"on_update": [], "on_wait": chunk},
                        })
                    si = dict(si)
                    si["on_wait"] = chunks[-1]
                    inst = dict(inst)
                    inst["sync_info"] = si
                out.append(inst)
            blk["instructions"] = out
    return changed


_orig_to_json_bytes = bass.Bass.to_json_bytes


def _patched_to_json_bytes(self) -> bytes:
    raw = _orig_to_json_bytes(self)
    j = orjson.loads(raw)
    if _split_excess_waits(j):
        return orjson.dumps(j)
    return raw


bass.Bass.to_json_bytes = _patched_to_json_bytes

# ---------------------------------------------------------------------------
# Problem constants (hardcoded; kernel.py must be self-contained)
# ---------------------------------------------------------------------------
B = 8            # batch = number of cores
N = 4096         # points per cloud
KNN = 20         # neighbors
CH = 64          # hidden channels
EPS = 1e-5
ALPHA = 0.2      # leaky-relu slope
NM = N // 128    # 32 row-tiles
NJ2 = KNN // 2   # 10 neighbor pairs
CNT = B * N * KNN  # batchnorm population size (global over all cores)
NEG = -1.0e30
PGROUPS = [(0, 4), (4, 8), (8, 10)]     # j2 pair-groups per psum tile
WCHUNKS = [(0, 512), (512, 1024), (1024, 1280)]  # W2 rhs chunks per parity

# --- packed top-k constants -------------------------------------------------
# Scores are quantized per-row to an 11-bit field with the 12-bit global
# column index packed into the low mantissa bits: after the scalar-engine
# affine (score*beta_row + bias_row + MAGIC), fp32 rounding at exponent 23
# quantizes to integers; subtracting MAGIC and adding j/4096 yields
# packed = q + j/4096 in [2048, 4096) whose low 12 mantissa bits are j.
# beta_row = (S0/CCLAMP)*exp(-|p_i|^2/3) adapts the clamp window to the
# local point density (validated: max_row d24^2/exp(sq/3) = 0.157 < 0.22).
MAGIC = 12582912.0          # 1.5 * 2^23
BIAS0 = 4050.0
S0 = 2002.0
CCLAMP = 0.22
LNB0 = 9.116029692504942    # ln(S0 / CCLAMP)

f32 = mybir.dt.float32
f16 = mybir.dt.float16
u32 = mybir.dt.uint32
u16 = mybir.dt.uint16
i16 = mybir.dt.int16
i32 = mybir.dt.int32
ACTF = mybir.ActivationFunctionType


def _bcast_mid(ap, reps):
    """Insert a step-0 dim after the partition dim: [P, F] -> [P, reps, F]."""
    return bass.AP(ap.tensor, ap.offset,
                   [list(ap.ap[0]), [0, reps], list(ap.ap[1])])


def _build_program():
    nc = bass.Bass("TRN2", target_bir_lowering=False, debug=False,
                   num_devices=B)

    xb = nc.dram_tensor("xb", [4, N], f32, kind="ExternalInput")
    jvt = nc.dram_tensor("jv", [128, N], f32, kind="ExternalInput")
    sqd = nc.dram_tensor("sqd", [N, 1], f32)
    w1g = nc.dram_tensor("w1g", [80, NJ2 * 128], f16, kind="ExternalInput")
    pts16 = nc.dram_tensor("pts16", [N, 4], f16, kind="ExternalInput")
    w2t = nc.dram_tensor("w2t", [128, CH], f16, kind="ExternalInput")
    bn1g = nc.dram_tensor("bn1g", [CH, 1], f32, kind="ExternalInput")
    bn1b = nc.dram_tensor("bn1b", [CH, 1], f32, kind="ExternalInput")
    bn2g = nc.dram_tensor("bn2g", [CH, 1], f32, kind="ExternalInput")
    bn2b = nc.dram_tensor("bn2b", [CH, 1], f32, kind="ExternalInput")
    out_t = nc.dram_tensor("out", [CH, N], f32, kind="ExternalOutput")

    cc1_in = nc.dram_tensor("cc1_in", [128, 2], f32)
    cc1_out = nc.dram_tensor("cc1_out", [128, 2], f32, addr_space="Shared")
    cc2_in = nc.dram_tensor("cc2_in", [CH, 2], f32)
    cc2_out = nc.dram_tensor("cc2_out", [CH, 2], f32, addr_space="Shared")
    groups = [list(range(B))]

    with tile.TileContext(nc) as tc:
        const = tc.alloc_tile_pool(name="const", bufs=1)
        dramp = tc.alloc_tile_pool(name="dram", bufs=1, space="DRAM")
        abpool = tc.alloc_tile_pool(name="ab", bufs=1)

        # whole-kernel tensors
        w2t_sb = const.tile([128, CH], f16)
        g1_sb = const.tile([CH, 1], f32)
        b1in_sb = const.tile([CH, 1], f32)
        g2_sb = const.tile([CH, 1], f32)
        b2in_sb = const.tile([CH, 1], f32)
        h1p = const.tile([128, NM * NJ2 * 128], f16)   # stored h1_pre
        a1_sb = const.tile([128, 1], f32)
        b1_sb = const.tile([128, 1], f32)
        a2_sb = const.tile([CH, 1], f32)
        b2_sb = const.tile([CH, 1], f32)
        s2h = const.tile([128, NM], f32)               # phaseC sum(h1a) slots
        s2q = const.tile([CH, NM * 2], f32)            # phaseC sum-sq slots
        jv_sb = const.tile([128, N], f32)              # j/4096 index column vec

        # phase A/B tensors (released after phase AB)
        # K=12 concatenated split operands: [hi;hi;lo] x [hi;lo;hi] computes
        # hi*hi + hi*lo + lo*hi in ONE matmul (fp32 to ~2^-24 rel)
        lhs_cat = abpool.tile([12, N], f16)
        rhs_cat = abpool.tile([12, N], f16)
        w1g_sb = abpool.tile([80, NJ2 * 128], f16)
        s1sum = abpool.tile([128, 3 * NM], f32)
        s1sq = abpool.tile([128, 3 * NM], f32)

        nc.sync.dma_start(w1g_sb[:], w1g.ap())
        nc.sync.dma_start(w2t_sb[:], w2t.ap())
        nc.sync.dma_start(g1_sb[:], bn1g.ap())
        nc.sync.dma_start(b1in_sb[:], bn1b.ap())
        nc.sync.dma_start(g2_sb[:], bn2g.ap())
        nc.sync.dma_start(b2in_sb[:], bn2b.ap())
        nc.sync.dma_start(jv_sb[:], jvt.ap())

        with nc.named_scope("stage0"):
            with tc.tile_pool(name="s0", bufs=1) as s0big, \
                 tc.tile_pool(name="s0s", bufs=1) as s0pool, \
                 tc.tile_pool(name="s0ps", bufs=2, space="PSUM") as s0psum:
                lhs_all = s0big.tile([4, N], f32, tag="lhs")
                rhs_all = s0big.tile([4, N], f32, tag="rhs")
                nc.sync.dma_start(lhs_all[:], xb.ap())
                nc.scalar.mul(rhs_all[0:3, :], lhs_all[0:3, :], 2.0)
                xsq = s0big.tile([4, N], f32, tag="scratch")
                nc.scalar.square(xsq[0:3, :], lhs_all[0:3, :])
                ones3 = nc.const_aps.tensor(1.0, (3, 1), f32)
                for j in range(N // 512):
                    ps = s0psum.tile([1, 512], f32, space="PSUM", tag="sq")
                    nc.tensor.matmul(ps[:], lhsT=ones3,
                                     rhs=xsq[0:3, bass.ts(j, 512)],
                                     start=True, stop=True)
                    sqneg = s0pool.tile([1, 512], f32, tag="sqneg")
                    nc.scalar.activation(sqneg[:], ps[:], ACTF.Copy, scale=-1.0)
                    nc.sync.dma_start(rhs_all[3:4, bass.ts(j, 512)], sqneg[:])
                nc.sync.dma_start(sqd.ap(), rhs_all[3:4, :])
                # fp16 hi/lo split of lhs/rhs for single-pass PE matmuls
                for full, cat, pattern in ((lhs_all, lhs_cat, (0, 0, 1)),
                                           (rhs_all, rhs_cat, (0, 1, 0))):
                    hi = s0pool.tile([4, N], f16, tag="hi")
                    nc.scalar.copy(hi[:], full[:])
                    rb = s0big.tile([4, N], f32, tag="scratch")
                    nc.vector.tensor_sub(rb[:], full[:], hi[:])
                    lo = s0pool.tile([4, N], f16, tag="lo")
                    nc.scalar.copy(lo[:], rb[:])
                    for slot, which in enumerate(pattern):
                        nc.sync.dma_start(cat[slot * 4:(slot + 1) * 4, :],
                                          (hi if which == 0 else lo)[:])

        # ------------------------------------------------------------------
        # Phases A (scores + top-k) and B (gather+add, transpose, BN1 stats)
        # ------------------------------------------------------------------
        with nc.named_scope("phaseAB"), \
             tc.tile_pool(name="scps", bufs=3, space="PSUM") as scps, \
             tc.tile_pool(name="score", bufs=3) as scorep, \
             tc.tile_pool(name="idxp", bufs=6) as idxp, \
             tc.tile_pool(name="gat", bufs=6) as gatp, \
             tc.tile_pool(name="trps", bufs=2, space="PSUM") as trps, \
             tc.tile_pool(name="dmy", bufs=2) as dmyp:
            # per-row packing params for ALL tiles at once:
            # beta = (S0/C)*exp(sqneg/3), bias = BIAS0+MAGIC+sqneg*beta
            sqn_all = idxp.tile([128, NM], f32, tag="sqna")
            nc.sync.dma_start(
                sqn_all[:],
                bass.AP(sqd.ap().tensor, 0, [[1, 128], [128, NM]]))
            beta_all = idxp.tile([128, NM], f32, tag="betaa")
            nc.scalar.activation(beta_all[:], sqn_all[:], ACTF.Exp,
                                 scale=1.0 / 3.0)
            nc.vector.tensor_scalar_mul(beta_all[:], beta_all[:], S0 / CCLAMP)
            biasm_all = idxp.tile([128, NM], f32, tag="biasma")
            nc.vector.tensor_mul(biasm_all[:], sqn_all[:], beta_all[:])
            nc.vector.tensor_scalar_add(biasm_all[:], biasm_all[:],
                                        BIAS0 + MAGIC)

            def emit_scores(m):
                beta = beta_all[:, m:m + 1]
                biasm = biasm_all[:, m:m + 1]
                score = scorep.tile([128, N], f32, tag="score")
                for j in range(N // 1024):
                    ps = scps.tile([128, 1024], f32, space="PSUM", tag="sc")
                    for h in range(2):
                        nc.tensor.matmul(
                            ps[:, h * 512:(h + 1) * 512],
                            lhsT=lhs_cat[:, bass.ts(m, 128)],
                            rhs=rhs_cat[:, j * 1024 + h * 512:
                                        j * 1024 + (h + 1) * 512],
                            start=True, stop=True)
                    # affine + integer-quantize (fp32 rounding at exp 23)
                    nc.scalar.activation(score[:, bass.ts(j, 1024)], ps[:],
                                         ACTF.Prelu, bias=biasm,
                                         scale=beta, alpha=1.0)
                # pack: score = (quantized - MAGIC) + j/4096
                for j in range(N // 1024):
                    nc.vector.scalar_tensor_tensor(
                        out=score[:, bass.ts(j, 1024)],
                        in0=score[:, bass.ts(j, 1024)], scalar=MAGIC,
                        in1=jv_sb[:, bass.ts(j, 1024)],
                        op0=mybir.AluOpType.subtract,
                        op1=mybir.AluOpType.add)
                return score

            pending = [emit_scores(0), emit_scores(1)]

            for m in range(NM):
                score = pending.pop(0) if pending else emit_scores(m)

                # L1: top-8 of each 256-col chunk (candidate values carry
                # their global column index in the low mantissa bits)
                cand = idxp.tile([128, 128], f32, tag="cand")
                for c in range(16):
                    nc.vector.max(out=cand[:, c * 8:(c + 1) * 8],
                                  in_=score[:, c * 256:(c + 1) * 256])
                # L2: top-24 of the candidates; col 0 is the self point
                sel = idxp.tile([128, 24], f32, tag="sel")
                dec = idxp.tile([128, 24], u32, tag="dec")
                # gather 8B neighbor coords (slot 0 = self, plain strided DMA)
                nbrP = gatp.tile([128, 128], f16, tag="nbrP")
                nc.sync.dma_start(nbrP[:, 0:4],
                                  pts16.ap()[m * 128:(m + 1) * 128, :])
                for r in range(3):
                    nc.vector.max(out=sel[:, r * 8:(r + 1) * 8], in_=cand[:])
                    if r < 2:
                        nc.vector.match_replace(
                            out=cand[:], in_to_replace=sel[:, r * 8:(r + 1) * 8],
                            in_values=cand[:], imm_value=NEG)
                    # decode this round's indices and issue its gathers so
                    # the Pool engine starts while later rounds still run
                    nc.vector.tensor_scalar(
                        out=dec[:, r * 8:(r + 1) * 8].bitcast(i32),
                        in0=sel[:, r * 8:(r + 1) * 8].bitcast(i32),
                        scalar1=4095, scalar2=None,
                        op0=mybir.AluOpType.bitwise_and)
                    for kk in range(max(1, r * 8), min(KNN, (r + 1) * 8)):
                        nc.gpsimd.indirect_dma_start(
                            out=nbrP[:, kk * 4:(kk + 1) * 4], out_offset=None,
                            in_=pts16.ap(),
                            in_offset=bass.IndirectOffsetOnAxis(
                                ap=dec[:, kk:kk + 1], axis=0))
                # emit the next score tile BEFORE the Pool-dependent
                # transpose/pair-matmuls: keeps the PE/ACT/DVE streams for
                # tile m+2 from queueing behind work that waits on this
                # tile's gathers (in-order engine queues)
                if m + 2 < NM:
                    pending.append(emit_scores(m + 2))
                nbrPT = gatp.tile([128, 128], f16, tag="nbrPT")
                nc.sync.dma_start_transpose(nbrPT[:], nbrP[:])
                # h1_pre pair blocks via K=8 (+K=4 self/C term) matmuls:
                # rows 0-63 = W1a@p_even + C, rows 64-127 = W1a@p_odd + C
                for gi, (j2a, j2b) in enumerate(PGROUPS):
                    gw = (j2b - j2a) * 128
                    trp = trps.tile([128, 512], f32, space="PSUM", tag="tr")
                    for g in range(j2b - j2a):
                        jp = j2a + g
                        nc.tensor.matmul(
                            trp[:, g * 128:(g + 1) * 128],
                            lhsT=w1g_sb[:, jp * 128:(jp + 1) * 128],
                            rhs=nbrPT[0:80, :],
                            start=True, stop=True)
                    col = (m * NJ2 + j2a) * 128
                    scol = m * 3 + gi
                    nc.scalar.activation(h1p[:, col:col + gw], trp[:, 0:gw],
                                         ACTF.Copy,
                                         accum_out=s1sum[:, scol:scol + 1])
                    dmy = dmyp.tile([128, 512], f32, tag="dmy")
                    nc.scalar.activation(dmy[:, 0:gw], trp[:, 0:gw],
                                         ACTF.Square,
                                         accum_out=s1sq[:, scol:scol + 1])

        # ------------------------------------------------------------------
        # BN1: global stats -> a1, b1
        # ------------------------------------------------------------------
        with nc.named_scope("bn1"), tc.tile_pool(name="bn1p", bufs=1) as bnp:
            st1 = bnp.tile([128, 2], f32)
            nc.vector.tensor_reduce(out=st1[:, 0:1], in_=s1sum[:],
                                    axis=mybir.AxisListType.X,
                                    op=mybir.AluOpType.add)
            nc.vector.tensor_reduce(out=st1[:, 1:2], in_=s1sq[:],
                                    axis=mybir.AxisListType.X,
                                    op=mybir.AluOpType.add)
            nc.sync.dma_start(cc1_in.ap(), st1[:])
            nc.gpsimd.collective_compute(
                kind="AllReduce", op=mybir.AluOpType.add,
                replica_groups=groups, ins=[cc1_in.ap()], outs=[cc1_out.ap()])
            st1g = bnp.tile([128, 2], f32)
            nc.sync.dma_start(st1g[:], cc1_out.ap())
            st1hi = bnp.tile([CH, 2], f32)
            nc.sync.dma_start(st1hi[:], st1g[CH:128, :])
            tot1 = bnp.tile([CH, 2], f32)
            nc.vector.tensor_add(tot1[:], st1g[0:CH, :], st1hi[:])
            mex = bnp.tile([CH, 2], f32)
            nc.scalar.mul(mex[:], tot1[:], 1.0 / CNT)
            mean1 = mex[:, 0:1]
            msq = bnp.tile([CH, 1], f32)
            nc.scalar.square(msq[:], mean1)
            var1 = bnp.tile([CH, 1], f32)
            nc.vector.tensor_sub(var1[:], mex[:, 1:2], msq[:])
            nc.scalar.activation(var1[:], var1[:], ACTF.Copy, bias=EPS)
            rcp1 = bnp.tile([CH, 1], f32)
            nc.vector.reciprocal(rcp1[:], var1[:])
            rs1 = bnp.tile([CH, 1], f32)
            nc.scalar.sqrt(rs1[:], rcp1[:])
            a1h = bnp.tile([CH, 1], f32)
            nc.vector.tensor_mul(a1h[:], rs1[:], g1_sb[:])
            am = bnp.tile([CH, 1], f32)
            nc.vector.tensor_mul(am[:], a1h[:], mean1)
            b1h = bnp.tile([CH, 1], f32)
            nc.vector.tensor_sub(b1h[:], b1in_sb[:], am[:])
            nc.sync.dma_start(a1_sb[0:CH, :], a1h[:])
            nc.sync.dma_start(a1_sb[CH:128, :], a1h[:])
            nc.sync.dma_start(b1_sb[0:CH, :], b1h[:])
            nc.sync.dma_start(b1_sb[CH:128, :], b1h[:])

        # ------------------------------------------------------------------
        # Phase C: h2_pre = W2 @ lrelu(a1*h1_pre + b1); BN2 stats (bn_stats)
        # ------------------------------------------------------------------
        cdpool = tc.alloc_tile_pool(name="cd", bufs=1)
        pooled_all = cdpool.tile([CH, N], f32)
        with nc.named_scope("phaseC"), \
             tc.tile_pool(name="h1a", bufs=2) as h1ap, \
             tc.tile_pool(name="pmx", bufs=4) as pmxp, \
             tc.tile_pool(name="h2ps", bufs=2, space="PSUM") as h2ps:
            for m in range(NM):
                mcol = m * NJ2 * 128
                h1a = h1ap.tile([128, NJ2 * 128], f16, tag="h1a")
                nc.scalar.activation(h1a[:], h1p[:, mcol:mcol + NJ2 * 128],
                                     ACTF.Prelu, bias=b1_sb[:, 0:1],
                                     scale=a1_sb[:, 0:1], alpha=ALPHA,
                                     accum_out=s2h[:, m:m + 1])
                # per-parity [64,1280] PSUM region; BN2 sums ride the
                # scalar-engine psum->sbuf copy (Copy/Square accum_out),
                # freeing the vector engine of bn_stats entirely
                pmaxes = []
                for par in range(2):
                    hp = h2ps.tile([CH, 3 * 512], f32, space="PSUM", tag="h2")
                    for c0, c1 in ((0, 512), (512, 1024), (1024, 1280)):
                        nc.tensor.matmul(
                            hp[:, c0:c1],
                            lhsT=w2t_sb[par * CH:(par + 1) * CH, :],
                            rhs=h1a[par * CH:(par + 1) * CH, c0:c1],
                            start=True, stop=True)
                    slot = m * 2 + par
                    dmy2 = pmxp.tile([CH, NJ2 * 128], f16, tag="dmy2")
                    nc.scalar.activation(dmy2[:], hp[:, 0:1280], ACTF.Square,
                                         accum_out=s2q[:, slot:slot + 1])
                    pm = pmxp.tile([CH, 128], f32, tag="pm")
                    hb = hp[:]
                    rm_in = bass.AP(hb.tensor, hb.offset,
                                    [list(hb.ap[0]), [1, 128], [128, NJ2]])
                    nc.vector.tensor_reduce(
                        out=pm[:], in_=rm_in,
                        axis=mybir.AxisListType.X, op=mybir.AluOpType.max)
                    pmaxes.append(pm)
                nc.vector.tensor_tensor(
                    out=pooled_all[:, bass.ts(m, 128)], in0=pmaxes[0][:],
                    in1=pmaxes[1][:], op=mybir.AluOpType.max)

        # ------------------------------------------------------------------
        # BN2: aggregate + global stats -> a2, b2
        # ------------------------------------------------------------------
        with nc.named_scope("bn2"), tc.tile_pool(name="bn2p", bufs=1) as bnp, \
             tc.tile_pool(name="bn2ps", bufs=1, space="PSUM") as bn2ps:
            # sum(h2) via linearity: sum_cols h2 = W2f @ row-fold(sum h1a)
            sh = bnp.tile([128, 1], f32)
            nc.vector.tensor_reduce(out=sh[:], in_=s2h[:],
                                    axis=mybir.AxisListType.X,
                                    op=mybir.AluOpType.add)
            shh = bnp.tile([CH, 1], f32)
            nc.sync.dma_start(shh[:], sh[CH:128, :])
            mh = bnp.tile([CH, 1], f32)
            nc.vector.tensor_add(mh[:], sh[0:CH, :], shh[:])
            mh16 = bnp.tile([CH, 1], f16)
            nc.vector.tensor_copy(mh16[:], mh[:])
            sps = bn2ps.tile([CH, 1], f32, space="PSUM")
            nc.tensor.matmul(sps[:], lhsT=w2t_sb[0:CH, :], rhs=mh16[:],
                             start=True, stop=True)
            st2 = bnp.tile([CH, 2], f32)
            nc.scalar.activation(st2[:, 0:1], sps[:], ACTF.Copy,
                                 scale=B / float(CNT))
            nc.vector.tensor_reduce(out=st2[:, 1:2], in_=s2q[:],
                                    axis=mybir.AxisListType.X,
                                    op=mybir.AluOpType.add)
            nc.scalar.mul(st2[:, 1:2], st2[:, 1:2], B / float(CNT))
            nc.sync.dma_start(cc2_in.ap(), st2[:])
            nc.gpsimd.collective_compute(
                kind="AllReduce", op=mybir.AluOpType.add,
                replica_groups=groups, ins=[cc2_in.ap()], outs=[cc2_out.ap()])
            tot2 = bnp.tile([CH, 2], f32)
            nc.sync.dma_start(tot2[:], cc2_out.ap())
            mean2 = bnp.tile([CH, 1], f32)
            nc.scalar.mul(mean2[:], tot2[:, 0:1], 1.0 / B)
            ex2b = bnp.tile([CH, 1], f32)
            nc.scalar.mul(ex2b[:], tot2[:, 1:2], 1.0 / B)
            msq2b = bnp.tile([CH, 1], f32)
            nc.scalar.square(msq2b[:], mean2[:])
            var2 = bnp.tile([CH, 1], f32)
            nc.vector.tensor_sub(var2[:], ex2b[:], msq2b[:])
            nc.scalar.activation(var2[:], var2[:], ACTF.Copy, bias=EPS)
            rcp2 = bnp.tile([CH, 1], f32)
            nc.vector.reciprocal(rcp2[:], var2[:])
            rs2 = bnp.tile([CH, 1], f32)
            nc.scalar.sqrt(rs2[:], rcp2[:])
            nc.vector.tensor_mul(a2_sb[:], rs2[:], g2_sb[:])
            am2 = bnp.tile([CH, 1], f32)
            nc.vector.tensor_mul(am2[:], a2_sb[:], mean2[:])
            nc.vector.tensor_sub(b2_sb[:], b2in_sb[:], am2[:])

        # ------------------------------------------------------------------
        # Phase D: recompute h2, apply BN2 + lrelu, max-pool over neighbors
        # ------------------------------------------------------------------
        with nc.named_scope("phaseD"), \
             tc.tile_pool(name="runm", bufs=4) as runp:
            for m in range(NM):
                runmax = runp.tile([CH, 128], f32, tag="run")
                nc.scalar.activation(runmax[:], pooled_all[:, bass.ts(m, 128)],
                                     ACTF.Prelu, bias=b2_sb[:, 0:1],
                                     scale=a2_sb[:, 0:1], alpha=ALPHA)
                nc.sync.dma_start(out_t.ap()[:, bass.ts(m, 128)], runmax[:])

        cdpool.release()
        abpool.release()
        const.release()
        dramp.release()

    return nc


_prog_cache = {}


def _get_program():
    if "nc" not in _prog_cache:
        _prog_cache["nc"] = _build_program()
    return _prog_cache["nc"]


def make_in_maps(x, W1, gamma1, beta1, W2, gamma2, beta2):
    x = np.asarray(x, dtype=np.float32)
    W1 = np.asarray(W1, dtype=np.float32)
    W2 = np.asarray(W2, dtype=np.float32)
    W1aT = W1[:, 0:3].T                      # [3, 64]
    W1cT = (W1[:, 3:6] - W1[:, 0:3]).T
    w1g = np.zeros((80, 1280), np.float32)
    for jp in range(10):
        blk = w1g[:, jp * 128:(jp + 1) * 128]
        blk[8 * jp:8 * jp + 3, 0:64] += W1aT
        blk[8 * jp + 4:8 * jp + 7, 64:128] += W1aT
        blk[0:3, 0:64] += W1cT
        blk[0:3, 64:128] += W1cT
    w1g = w1g.astype(np.float16)
    g2 = np.asarray(gamma2, dtype=np.float32).reshape(CH)
    sgn2 = np.where(g2 < 0, -1.0, 1.0).astype(np.float32)
    W2f = W2 * sgn2[:, None]          # flip rows so the BN2 scale is >= 0
    w2t_1 = np.ascontiguousarray(W2f.T).astype(np.float16)    # [64, 64]
    w2t = np.concatenate([w2t_1, w2t_1], axis=0)              # [128, 64]
    col = lambda v: np.ascontiguousarray(
        np.asarray(v, dtype=np.float32).reshape(CH, 1))
    jv = np.ascontiguousarray(np.broadcast_to(
        (np.arange(N, dtype=np.float32) / 4096.0), (128, N))).astype(np.float32)
    return [{
        "xb": np.concatenate([x[b], np.ones((1, N), np.float32)], axis=0),
        "pts16": np.concatenate([x[b].T.astype(np.float16),
                                 np.ones((N, 1), np.float16)], axis=1),
        "jv": jv,
        "w1g": w1g, "w2t": w2t,
        "bn1g": col(gamma1), "bn1b": col(beta1),
        "bn2g": col(np.abs(g2)), "bn2b": col(beta2),
    } for b in range(B)]


def kernel(x, W1, gamma1, beta1, W2, gamma2, beta2):
    nc = _get_program()
    in_maps = make_in_maps(x, W1, gamma1, beta1, W2, gamma2, beta2)
    res = run_bass_kernel_spmd(nc, in_maps, list(range(B)))
    out = np.stack([res.results[b]["out"] for b in range(B)], axis=0)
    return out.astype(np.float32)

